# revision 2
# baseline (speedup 1.0000x reference)
"""Trainium2 Bass kernel v3.3 for the pre-LN decoder block.

Data-parallel over batch across 8 cores (8 batches/core), weights replicated.

Deltas vs the v2 baseline:
- zero activation-table reloads: scalar engine runs only {Exp, Relu, Copy};
  LN rstd is computed on the vector engine (reciprocal + rsqrt-Newton).
- Wo runs as fp8e4m3 DoubleRow over head-pairs 0/1 (attnT is written fp8 by
  the softmax-normalize multiply, so the fp8 path costs no extra ops).
- DMA transposes (the h/h2 [t,d]->[d,t] xbar moves, ~1.2us of sequencer
  dispatch each) are split across the SP and ACT queues instead of
  serializing on SP.
- elementwise work is spread by measured engine rates: masks on gpsimd,
  relu+residual+casts balanced between scalar and vector, denominator
  reciprocals batched per head-pair into one [65,T] op.
- x input is bf16 (host cast), softmax denominators via the aug-ones column.
- per-iteration emission interleaves MLP(b-1) blocks between attention(b)
  stages so every engine queue always has ready work.
"""

import math

import numpy as np
import ml_dtypes

import concourse.bass as bass  # noqa: F401
import concourse.bacc as bacc
import concourse.mybir as mybir
import concourse.tile as tile
from concourse.bass_utils import run_bass_kernel_spmd

F32 = mybir.dt.float32
BF16 = mybir.dt.bfloat16
FP8 = mybir.dt.float8e4

B, T, D = 64, 512, 384
H, HS = 6, 64
NCORES = 8
NB = B // NCORES
FF = 4 * D
EPS = 1e-5
SCALE = 1.0 / math.sqrt(D)
NT = T // 128
KD = D // 128
KH = FF // 128
NPAIR = H // 2
P = 128
DDR = 256
DR = mybir.MatmulPerfMode.DoubleRow


def _emit(nc, tc, ctx, x, wqkv, wo_dr, wo_tl, w1, w2, bb1, out, nb=NB,
          zero_bias=True, bo=None, bb2=None, dbg=None):
    # ---------------- pools ----------------
    wp = ctx.enter_context(tc.tile_pool(name="weights", bufs=1))
    xp = ctx.enter_context(tc.tile_pool(name="xres", bufs=3))
    x2p = ctx.enter_context(tc.tile_pool(name="x2res", bufs=12))
    h8p = ctx.enter_context(tc.tile_pool(name="h8", bufs=2))
    htp = ctx.enter_context(tc.tile_pool(name="hT", bufs=2))
    qkp = ctx.enter_context(tc.tile_pool(name="qk", bufs=2))
    vp = ctx.enter_context(tc.tile_pool(name="vaug", bufs=2))
    ep = ctx.enter_context(tc.tile_pool(name="expT", bufs=3))
    rp = ctx.enter_context(tc.tile_pool(name="rec", bufs=2))
    ap_ = ctx.enter_context(tc.tile_pool(name="attnT", bufs=2))
    mp = ctx.enter_context(tc.tile_pool(name="mlp", bufs=2))
    sp = ctx.enter_context(tc.tile_pool(name="stats", bufs=2))

    pp512 = ctx.enter_context(tc.tile_pool(name="pp512", bufs=3, space="PSUM"))
    ppat = ctx.enter_context(tc.tile_pool(name="ppat", bufs=2, space="PSUM"))
    pp384 = ctx.enter_context(tc.tile_pool(name="pp384", bufs=3, space="PSUM"))

    # ---------------- constants ----------------
    masku = wp.tile([P, P], BF16, tag="masku", name="masku")
    nc.gpsimd.memset(masku, 1.0)
    nc.gpsimd.affine_select(
        out=masku, in_=masku,
        compare_op=mybir.AluOpType.is_ge,
        fill=0.0, base=0,
        pattern=[[1, P]], channel_multiplier=-1,
    )

    # ---------------- weights -> SBUF ----------------
    wqkv_sb = [[wp.tile([P, H * HS], BF16, tag=f"wqkv{i}_{kd}",
                        name=f"wqkv{i}_{kd}") for kd in range(KD)]
               for i in range(3)]
    wo_dr_sb = wp.tile([P, 2, D], FP8, tag="wodr", name="wodr")
    wo_tl_sb = wp.tile([P, D], BF16, tag="wotl", name="wotl")
    w1_sb = [wp.tile([P, FF], BF16, tag=f"w1{kd}", name=f"w1{kd}")
             for kd in range(KD)]
    w2_sb = [wp.tile([P, D], BF16, tag=f"w2{kh}", name=f"w2{kh}")
             for kh in range(KH)]
    bb1_sb = wp.tile([P, KH], F32, tag="bb1", name="bb1")

    def load_weights_front():
        for i in range(3):
            for kd in range(KD):
                nc.sync.dma_start(out=wqkv_sb[i][kd], in_=wqkv[i, kd])

    def load_weights_back():
        nc.sync.dma_start(out=wo_dr_sb, in_=wo_dr)
        nc.sync.dma_start(out=wo_tl_sb, in_=wo_tl)
        for kd in range(KD):
            nc.sync.dma_start(out=w1_sb[kd], in_=w1[kd])
        for kh in range(KH):
            nc.sync.dma_start(out=w2_sb[kh], in_=w2[kh])
        nc.sync.dma_start(out=bb1_sb, in_=bb1)

    if not zero_bias:
        ones1 = wp.tile([1, P], BF16, tag="ones1", name="ones1")
        nc.vector.memset(ones1, 1.0)
        bo_sb = wp.tile([1, D], BF16, tag="bo", name="bo")
        nc.sync.dma_start(out=bo_sb, in_=bo)
        bb2_sb = wp.tile([1, D], BF16, tag="bb2", name="bb2")
        nc.sync.dma_start(out=bb2_sb, in_=bb2)

    # ---------------- state ----------------
    x_tiles = {}
    x2_tiles = {}
    hT = {}          # b -> 3 tiles [128, T] bf16
    h2T = {}
    qT = {}
    kT = {}
    ets = {}
    vaug = {}
    pats = {}
    attnT_dr = {}    # b -> [128, 2, T] fp8 (pairs 0,1)
    attnT_tl = {}    # b -> [128, T] bf16  (pair 2)
    rT = {}

    def load_x(b):
        xb = xp.tile([P, NT, D], BF16, tag="x", name="x")
        nc.sync.dma_start(
            out=xb, in_=x[b].rearrange("(a b) c -> b a c", b=P))
        x_tiles[b] = [xb[:, tt, :] for tt in range(NT)]

    def rstd_chain(mv, pfx, n=NT):
        """rstd = rsqrt(var+eps) = r*z, r = recip(var+eps), z = rsqrt(r)
        via 3 Newton steps from the linear seed (3-r)/2. Vector only."""
        ve = sp.tile([P, n], F32, tag=f"{pfx}_ve", name=f"{pfx}_ve")
        nc.vector.tensor_scalar_add(out=ve, in0=mv[:, :, 1], scalar1=EPS)
        r = sp.tile([P, n], F32, tag=f"{pfx}_r", name=f"{pfx}_r")
        nc.vector.reciprocal_approx_fast(out=r, in_=ve)
        z = sp.tile([P, n], F32, tag=f"{pfx}_z", name=f"{pfx}_z")
        nc.vector.tensor_scalar(out=z, in0=r, scalar1=-0.5, scalar2=1.5,
                                op0=mybir.AluOpType.mult,
                                op1=mybir.AluOpType.add)
        t1 = sp.tile([P, n], F32, tag=f"{pfx}_t1", name=f"{pfx}_t1")
        for _ in range(3):
            nc.vector.tensor_mul(out=t1, in0=z, in1=z)
            nc.vector.tensor_mul(out=t1, in0=t1, in1=r)
            nc.vector.tensor_scalar(out=t1, in0=t1, scalar1=-0.5,
                                    scalar2=1.5,
                                    op0=mybir.AluOpType.mult,
                                    op1=mybir.AluOpType.add)
            nc.vector.tensor_mul(out=z, in0=z, in1=t1)
        rstd = sp.tile([P, n], F32, tag=f"{pfx}_rstd", name=f"{pfx}_rstd")
        nc.vector.tensor_mul(out=rstd, in0=r, in1=z)
        return rstd

    def ln_stats(tiles, pfx):
        stats = sp.tile([P, NT, 6], F32, tag=f"{pfx}_st", name=f"{pfx}_st")
        mv = sp.tile([P, NT, 2], F32, tag=f"{pfx}_mv", name=f"{pfx}_mv")
        for tt in range(NT):
            nc.vector.bn_stats(out=stats[:, tt, :], in_=tiles[tt])
            nc.vector.bn_aggr(out=mv[:, tt, :], in_=stats[:, tt, :])
        return mv, rstd_chain(mv, pfx)

    def ln_norm(src_tiles, pfx):
        """LN over 4 [128, D] tiles -> 4 normalized bf16 h tiles."""
        mv, rstd = ln_stats(src_tiles, pfx)
        hhs = []
        for tt in range(NT):
            hh = h8p.tile([P, D], BF16, tag=f"{pfx}h{tt}", name=f"{pfx}h{tt}")
            nc.vector.tensor_scalar(out=hh, in0=src_tiles[tt],
                                    scalar1=mv[:, tt, 0:1],
                                    scalar2=rstd[:, tt:tt + 1],
                                    op0=mybir.AluOpType.subtract,
                                    op1=mybir.AluOpType.mult)
            hhs.append(hh)
        return hhs

    def ln_tp(hhs, dst_T):
        for tt in range(NT):
            for kd in range(KD):
                nc.sync.dma_start_transpose(
                    out=dst_T[kd][:, tt * P:(tt + 1) * P],
                    in_=hhs[tt][:, kd * P:(kd + 1) * P])

    h2n = {}

    def ln1(b):
        hT[b] = [htp.tile([P, T], BF16, tag=f"hT{kd}", name=f"hT{kd}")
                 for kd in range(KD)]
        ln_tp(ln_norm(x_tiles[b], "ln1"), hT[b])

    def ln2_norm(b):
        h2n[b] = ln_norm(x2_tiles[b], "ln2")

    def ln2_tp(b):
        h2T[b] = [htp.tile([P, T], BF16, tag=f"h2T{kd}", name=f"h2T{kd}")
                  for kd in range(KD)]
        ln_tp(h2n[b], h2T[b])
        del h2n[b]

    def qk(b):
        for p in range(NPAIR):
            for i, store, nm in ((0, qT, "qT"), (1, kT, "kT")):
                ps = pp512.tile([P, T], F32, tag="p512", name="ps512")
                for kd in range(KD):
                    nc.tensor.matmul(ps, wqkv_sb[i][kd][:, p * P:(p + 1) * P],
                                     hT[b][kd], start=(kd == 0),
                                     stop=(kd == KD - 1))
                sb = qkp.tile([P, T], BF16, tag=f"{nm}{p}", name=f"{nm}{p}")
                nc.vector.tensor_copy(out=sb, in_=ps)
                store[(b, p)] = sb
                if dbg is not None and b == 0:
                    nc.sync.dma_start(out=dbg[nm][p * P:(p + 1) * P, :],
                                      in_=sb)

    def vproj(b):
        for ts in range(NT):
            ps = pp384.tile([P, D], F32, tag="p384", name="ps384",
                            padded_shape=[P, 512])
            for kd in range(KD):
                nc.tensor.matmul(ps, hT[b][kd][:, ts * P:(ts + 1) * P],
                                 wqkv_sb[2][kd], start=(kd == 0),
                                 stop=(kd == KD - 1))
            va = vp.tile([P, H, HS + 1], BF16, tag=f"vaug{ts}",
                         name=f"vaug{ts}")
            nc.vector.memset(va[:, :, HS:HS + 1], 1.0)
            if dbg is not None and b == 0:
                vtmp = h8p.tile([P, D], F32, tag="vdbg", name="vdbg")
                nc.vector.tensor_copy(out=vtmp, in_=ps)
                nc.sync.dma_start(out=dbg["v"][ts * P:(ts + 1) * P, :],
                                  in_=vtmp)
            nc.vector.tensor_copy(
                out=va[:, :, 0:HS],
                in_=ps.rearrange("p (h e) -> p h e", h=H))
            vaug[(b, ts)] = va

    def scores(b, p):
        qk_q, qk_k = qT[(b, p)], kT[(b, p)]

        def emit_sc(q, ts):
            esl = slice(q * HS, (q + 1) * HS)
            ncols = T - ts * P
            psc = pp512.tile([P, T], F32, tag="p512", name="ps512")
            nc.tensor.matmul(psc[:, 0:ncols],
                             qk_k[esl, ts * P:(ts + 1) * P],
                             qk_q[esl, ts * P:T],
                             start=True, stop=True)
            h = 2 * p + q
            et = ep.tile([P, T], BF16, tag=f"ets{q}{ts}",
                         name=f"ets{q}{ts}")
            nc.scalar.activation(out=et[:, 0:ncols], in_=psc[:, 0:ncols],
                                 func=mybir.ActivationFunctionType.Exp,
                                 scale=SCALE)
            nc.vector.tensor_mul(out=et[:, 0:P], in0=et[:, 0:P], in1=masku)
            ets[(b, h, ts)] = et

        emit_sc(0, 0)
        emit_sc(1, 0)
        emit_sc(0, 1)
        emit_sc(1, 1)
        emit_sc(0, 2)
        emit_sc(1, 2)
        emit_sc(0, 3)
        emit_sc(1, 3)

    def attnv(b, h):
        pa = ppat.tile([HS + 1, T], F32, tag="pat", name="pat")
        pats[(b, h)] = pa
        for ts in range(NT):
            ncols = T - ts * P
            nc.tensor.matmul(pa[:, ts * P:T], vaug[(b, ts)][:, h, :],
                             ets[(b, h, ts)][:, 0:ncols],
                             start=(ts == 0), stop=(ts == NT - 1),
                             skip_group_check=True)

    def alloc_attnT(b):
        attnT_dr[b] = ap_.tile([P, 2, T], FP8, tag="attnTdr", name="attnTdr")
        attnT_tl[b] = ap_.tile([P, T], BF16, tag="attnTtl", name="attnTtl")

    def att_norm(b, p):
        """normalize pair p's raw attn rows by the aug denominators.
        Both heads' denominators land on partitions 0 / 64 of one tile so a
        single [65, T] reciprocal covers them (rows 1..63 are junk)."""
        rrs = []
        for q in (0, 1):
            h = 2 * p + q
            dn = rp.tile([1, T], F32, tag=f"dnm{q}", name=f"dnm{q}")
            nc.scalar.copy(out=dn, in_=pats[(b, h)][HS:HS + 1, :])
            rr = rp.tile([1, T], F32, tag=f"rr{q}", name=f"rr{q}")
            nc.vector.reciprocal_approx_fast(out=rr, in_=dn)
            rrs.append(rr)
        for q in (0, 1):
            h = 2 * p + q
            rbc = rp.tile([HS, T], F32, tag=f"rbc{q}", name=f"rbc{q}")
            nc.gpsimd.partition_broadcast(out_ap=rbc, in_ap=rrs[q])
            if p < 2:
                dst = attnT_dr[b][q * HS:(q + 1) * HS, p, :]
            else:
                dst = attnT_tl[b][q * HS:(q + 1) * HS, :]
            nc.vector.tensor_mul(out=dst, in0=pats[(b, h)][0:HS, :], in1=rbc)
            if dbg is not None and b == 0:
                tmp = rp.tile([HS, T], F32, tag="dbgt", name="dbgt")
                nc.vector.tensor_copy(out=tmp, in_=dst)
                nc.sync.dma_start(out=dbg["attnT"][h * HS:(h + 1) * HS, :],
                                  in_=tmp)

    def wo_tt(b, tt):
        po = pp384.tile([P, D], F32, tag="p384", name="ps384",
                        padded_shape=[P, 512])
        lhs_dr = attnT_dr[b][:, :, tt * P:(tt + 1) * P]
        nc.tensor.matmul(po[:, 0:DDR], lhs_dr, wo_dr_sb[:, :, 0:DDR],
                         start=True, stop=False, perf_mode=DR,
                         skip_group_check=True)
        nc.tensor.matmul(po[:, DDR:D], lhs_dr, wo_dr_sb[:, :, DDR:D],
                         start=False, stop=False, perf_mode=DR,
                         skip_group_check=True)
        nc.tensor.matmul(po, attnT_tl[b][:, tt * P:(tt + 1) * P], wo_tl_sb,
                         start=False, stop=zero_bias,
                         skip_group_check=True)
        if not zero_bias:
            nc.tensor.matmul(po, ones1, bo_sb, start=False, stop=True,
                             skip_group_check=True)
        x2t = x2p.tile([P, D], F32, tag="x2", name="x2")
        nc.vector.tensor_add(out=x2t, in0=po, in1=x_tiles[b][tt])
        if b not in x2_tiles:
            x2_tiles[b] = [None] * NT
        x2_tiles[b][tt] = x2t

    def w1_block(m, grp, relu_scalar=None):
        """W1 + relu for kh in [3*grp, 3*grp+3)."""
        if m not in rT:
            rT[m] = [None] * KH
        for kh in range(3 * grp, 3 * grp + 3):
            pm = pp512.tile([P, T], F32, tag="p512", name="ps512")
            for kd in range(KD):
                nc.tensor.matmul(pm, w1_sb[kd][:, kh * P:(kh + 1) * P],
                                 h2T[m][kd], start=(kd == 0),
                                 stop=(kd == KD - 1))
            rt = mp.tile([P, T], BF16, tag=f"rT{kh}", name=f"rT{kh}")
            if relu_scalar if relu_scalar is not None else (kh % 2 == 0):
                nc.scalar.activation(out=rt, in_=pm,
                                     func=mybir.ActivationFunctionType.Relu,
                                     bias=bb1_sb[:, kh:kh + 1])
            else:
                nc.vector.tensor_scalar(out=rt, in0=pm,
                                        scalar1=bb1_sb[:, kh:kh + 1],
                                        scalar2=0.0,
                                        op0=mybir.AluOpType.add,
                                        op1=mybir.AluOpType.max)
            rT[m][kh] = rt

    def w2_tt(m, tt):
        po2 = pp384.tile([P, D], F32, tag="p384", name="ps384",
                         padded_shape=[P, 512])
        for kh in range(KH):
            nc.tensor.matmul(po2, rT[m][kh][:, tt * P:(tt + 1) * P],
                             w2_sb[kh], start=(kh == 0),
                             stop=(kh == KH - 1 and zero_bias))
        if not zero_bias:
            nc.tensor.matmul(po2, ones1, bb2_sb, start=False, stop=True)
        ot = mp.tile([P, D], F32, tag="ot", name="ot")
        nc.vector.tensor_add(out=ot, in0=po2, in1=x2_tiles[m][tt])
        nc.scalar.dma_start(out=out[m, tt * P:(tt + 1) * P, :], in_=ot)

    # ---------------- schedule ----------------
    load_x(0)
    load_weights_front()
    if nb > 1:
        load_x(1)
    ln1(0)
    load_weights_back()

    for b in range(nb):
        m = b - 2
        if b + 1 < nb:
            ln1(b + 1)
        qk(b)
        vproj(b)
        if b + 2 < nb:
            load_x(b + 2)
        if b - 1 >= 0:
            ln2_tp(b - 1)
        for p in range(NPAIR):
            scores(b, p)
            if m >= 0:
                w1_block(m, p)
        alloc_attnT(b)
        for p in range(NPAIR):
            attnv(b, 2 * p)
            attnv(b, 2 * p + 1)
            att_norm(b, p)
            if m >= 0 and p == 0:
                w1_block(m, 3)
            if m >= 0 and p < 2:
                w2_tt(m, p)
        if m >= 0:
            w2_tt(m, 2)
            w2_tt(m, 3)
        for tt in range(NT):
            wo_tt(b, tt)
        ln2_norm(b)
        del x_tiles[b]

    ln2_tp(nb - 1)
    if nb >= 2:
        for grp in range(4):
            w1_block(nb - 2, grp)
            w1_block(nb - 1, grp)
        for tt in range(NT):
            w2_tt(nb - 2, tt)
            w2_tt(nb - 1, tt)
    else:
        for grp in range(4):
            w1_block(nb - 1, grp)
        for tt in range(NT):
            w2_tt(nb - 1, tt)


def build(nb=NB, zero_bias=True, debug=False):
    from contextlib import ExitStack

    nc = bacc.Bacc("TRN2", target_bir_lowering=False, debug=False)
    x = nc.declare_dram_parameter("x", [nb, T, D], BF16, isOutput=False).ap()
    wqkv = nc.declare_dram_parameter("wqkv", [3, KD, P, H * HS], BF16,
                                     isOutput=False).ap()
    wo_dr = nc.declare_dram_parameter("wo_dr", [P, 2, D], FP8,
                                      isOutput=False).ap()
    wo_tl = nc.declare_dram_parameter("wo_tl", [P, D], BF16,
                                      isOutput=False).ap()
    w1 = nc.declare_dram_parameter("w1", [KD, P, FF], BF16, isOutput=False).ap()
    w2 = nc.declare_dram_parameter("w2", [KH, P, D], BF16, isOutput=False).ap()
    bb1 = nc.declare_dram_parameter("bb1", [P, KH], F32, isOutput=False).ap()
    bo = bb2 = None
    if not zero_bias:
        bo = nc.declare_dram_parameter("bo", [1, D], BF16, isOutput=False).ap()
        bb2 = nc.declare_dram_parameter("bb2", [1, D], BF16,
                                        isOutput=False).ap()
    out = nc.declare_dram_parameter("out", [nb, T, D], F32, isOutput=True).ap()
    dbg = None
    if debug:
        dbg = {
            "qT": nc.declare_dram_parameter("dbg_qT", [D, T], BF16,
                                            isOutput=True).ap(),
            "kT": nc.declare_dram_parameter("dbg_kT", [D, T], BF16,
                                            isOutput=True).ap(),
            "v": nc.declare_dram_parameter("dbg_v", [T, D], F32,
                                           isOutput=True).ap(),
            "attnT": nc.declare_dram_parameter("dbg_attnT", [D, T], F32,
                                               isOutput=True).ap(),
        }

    with tile.TileContext(nc) as tc:
        with ExitStack() as ctx:
            _emit(nc, tc, ctx, x, wqkv, wo_dr, wo_tl, w1, w2, bb1, out,
                  nb=nb, zero_bias=zero_bias, bo=bo, bb2=bb2, dbg=dbg)
    nc.compile()
    return nc


def _pack_inputs(inputs):
    bf = ml_dtypes.bfloat16
    f8 = ml_dtypes.float8_e4m3fn
    f = lambda k: np.asarray(inputs[k], np.float32)
    g1, b1v, g2, b2v = f("g1"), f("b1"), f("g2"), f("b2")
    assert np.abs(b1v).max() == 0.0 and np.abs(b2v).max() == 0.0, \
        "nonzero LN beta not supported"
    bo, bb2 = f("bo"), f("bb2")
    zero_bias = (np.abs(bo).max() == 0.0) and (np.abs(bb2).max() == 0.0)

    def qkv_flat(Wfull):
        return (Wfull * g1[None, :, None]).transpose(1, 0, 2).reshape(D, H * HS)

    wqkv = np.stack([
        qkv_flat(f("Wq")).reshape(KD, P, H * HS),
        qkv_flat(f("Wk")).reshape(KD, P, H * HS),
        qkv_flat(f("Wv")).reshape(KD, P, H * HS),
    ]).astype(bf)

    Wo = f("Wo")
    shared = {
        "wqkv": np.ascontiguousarray(wqkv),
        # wo_dr[ep, i, d] = Wo[i*128 + ep, d]  (head-pairs 0,1 as DR slots)
        "wo_dr": np.ascontiguousarray(
            Wo[:DDR].reshape(2, P, D).transpose(1, 0, 2).astype(f8)),
        "wo_tl": np.ascontiguousarray(Wo[DDR:].astype(bf)),
        "w1": (f("W1") * g2[:, None]).reshape(KD, P, FF).astype(bf),
        "w2": f("W2").reshape(KH, P, D).astype(bf),
        "bb1": np.ascontiguousarray(f("bb1").reshape(KH, P).T),
    }
    if not zero_bias:
        shared["bo"] = bo.reshape(1, D).astype(bf)
        shared["bb2"] = bb2.reshape(1, D).astype(bf)
    return shared, zero_bias


def run(inputs, trace=False, **kw):
    bf = ml_dtypes.bfloat16
    x = np.asarray(np.asarray(inputs["x"], np.float32), bf)
    shared, zero_bias = _pack_inputs(inputs)

    nc = build(zero_bias=zero_bias)
    in_maps = []
    for c in range(NCORES):
        mm = dict(shared)
        mm["x"] = np.ascontiguousarray(x[c * NB:(c + 1) * NB])
        in_maps.append(mm)
    res = run_bass_kernel_spmd(nc, in_maps, list(range(NCORES)), trace=trace,
                               **kw)
    outv = np.concatenate([r["out"] for r in res.results], axis=0)
    return outv, res


def kernel(**inputs):
    return run(inputs)[0]


if __name__ == "__main__":
    print("built", build())
